# revision 15
# baseline (speedup 1.0000x reference)
"""Trainium2 Bass kernel for CrossAttention (B=2, Sq=2048, Skv=4096, D=768, H=12).

Sharding: 8 cores = 2 batches x 4 head-groups (3 heads each). Each core
computes its 3 heads' attention and a partial output projection; the host
sums the 4 partial projections per batch and adds bo.

Device data flow (per core, matmul inputs bf16, accumulation fp32):
  - hidden^T / encoder^T shipped pre-transposed; DMAs ordered so the score
    pipeline starts as early as possible (wk_ab first, then the leading
    eT/hT columns).
  - Q^T/K^T computed in [dh(part), seq(free)] layout. Heads h0,h1 stacked on
    partition halves; h2 duplicated on both halves (weights shipped
    duplicated).
  - Scores S^T[kv, q] per 128-kv chunk, grouped into alternating
    [128,1536]/[128,1024] PSUM megas so ScalarE exp amortizes its startup.
  - All projection/producer work (K^T, Q^T, V, output projection) runs
    through two 256-element slots of a single persistent aux PSUM bank,
    ping-ponged so each unit's PSUM->SBUF copy overlaps the next unit's
    matmuls (chains stay sequential per bank; reads are unaffected by a
    later chain's start).
  - K^T_ab and V are produced on demand (pulled by the score/PV stream);
    the rest drains on a paced static schedule inside the windows.
  - PV: V augmented with a ones column (M=65) so PSUM accumulates ctx^T and
    the softmax denominator in one stream; PV issue deferred one mega so
    exp never waits behind PV in the PE queue.
  - Phase A slices stagger h1 one chunk behind h0 so the two finalizes
    (reciprocal + gpsimd broadcast + multiply) don't collide at the window
    boundary.
  - Output projection: ctx^T is exactly the lhsT needed; 192-wide quarters
    through the aux slots; partial out to DRAM.
"""

import numpy as np
import ml_dtypes

import concourse.bass as bass
import concourse.bacc as bacc
import concourse.mybir as mybir
from concourse.tile import TileContext
from concourse.bass_utils import run_bass_kernel_spmd

BF16 = mybir.dt.bfloat16
F32 = mybir.dt.float32
AF = mybir.ActivationFunctionType

B, SQ, SKV, D, H, DH = 2, 2048, 2048 * 2, 768, 12, 64
HPC = 3          # heads per core
NKC = 6          # contract chunks (768 / 128)
NKV = 32         # kv chunks (4096 / 128)
NQC = 4          # q chunks of 512
QC = 512
P = 128

# wgtA free-dim element offsets (bf16): weights needed first
O_WKAB = 0
O_WQAB = 768
O_WV3 = 1536
NWA = O_WV3 + NKC * 192      # 2688
# wgtB offsets: the rest
O_WQ2D = 0
O_WK2D = 768
O_WOAB = 1536
O_WO2 = O_WOAB + 768
NWB = O_WO2 + 768            # 3072
NE = NKC * SKV               # 24576 encoder^T
NH = NKC * SQ                # 12288 hidden^T


def _mega_layout(n):
    # alternating 3/2-slice megas covering n slices
    sizes = []
    left = n
    while left > 0:
        s = 3 if (len(sizes) % 2 == 0) else 2
        s = min(s, left)
        sizes.append(s)
        left -= s
    base = [0]
    for s in sizes:
        base.append(base[-1] + s)
    return sizes, base


LAST_RESULT = None  # BassKernelResults of the most recent run (for test.py)

_CACHED_NC = None

# Phase A slices: h0 runs one chunk ahead of h1 so their finalizes stagger.
SLICES_A = []
for _c in range(NKV):
    SLICES_A.append((0, _c, 0))
    if _c >= 1:
        SLICES_A.append((1, _c - 1, 1))
SLICES_A.append((1, NKV - 1, 1))
assert len(SLICES_A) == 64

# Phase B slices: head 2, chunk i, row-half alternating (duplicated tiles)
SLICES_B = [(2, _c, _c % 2) for _c in range(NKV)]


def _build_nc():
    nc = bacc.Bacc()

    wgtA = nc.declare_dram_parameter("wgtA", [P, NWA], BF16, isOutput=False)
    wgtB = nc.declare_dram_parameter("wgtB", [P, NWB], BF16, isOutput=False)
    biasf = nc.declare_dram_parameter("biasf", [P, 6], F32, isOutput=False)
    eTd = nc.declare_dram_parameter("eT", [P, NE], BF16, isOutput=False)
    hTd = nc.declare_dram_parameter("hT", [P, NH], BF16, isOutput=False)
    out = nc.declare_dram_parameter("out", [SQ, D], F32, isOutput=True)

    with TileContext(nc) as tc:
        with (
            tc.tile_pool(name="persist", bufs=1) as pp,
            tc.tile_pool(name="inb", bufs=1) as ip,
            tc.tile_pool(name="aux", bufs=1, space="PSUM") as auxp,
            tc.tile_pool(name="sT3", bufs=1, space="PSUM") as sT3p,
            tc.tile_pool(name="sT2", bufs=1, space="PSUM") as sT2p,
            tc.tile_pool(name="ctx", bufs=2, space="PSUM") as ctxp,
            tc.tile_pool(name="es", bufs=8) as esp,
            tc.tile_pool(name="fin", bufs=4) as finp,
            tc.tile_pool(name="ost", bufs=2) as ostp,
        ):
            qT_ab = pp.tile([P, SQ], BF16, tag="qT_ab")
            qT_2d = pp.tile([P, SQ], BF16, tag="qT_2d")
            kT_ab = pp.tile([P, SKV], BF16, tag="kT_ab")
            kT_2d = pp.tile([P, SKV], BF16, tag="kT_2d")
            vv = pp.tile([P, NKV, HPC, 65], BF16, tag="vv")
            ctxn_ab = pp.tile([P, SQ], BF16, tag="ctxn_ab")
            ctxn_2 = pp.tile([P, SQ], BF16, tag="ctxn_2")
            wgtA_sb = pp.tile([P, NWA], BF16, tag="wgtA")
            wgtB_sb = pp.tile([P, NWB], BF16, tag="wgtB")
            bias_sb = pp.tile([P, 6], F32, tag="biasf")
            warm = pp.tile([1, 2], F32, tag="warm")
            eT_sb = ip.tile([P, NKC, SKV], BF16, tag="eT")
            hT_sb = ip.tile([P, NKC, SQ], BF16, tag="hT")

            # Warm up the ScalarE exp table while DMAs run.
            nc.vector.memset(warm[0:1, 0:1], 0.0)
            nc.scalar.activation(warm[0:1, 1:2], warm[0:1, 0:1], AF.Exp)
            nc.vector.memset(vv[:, :, :, 64:65], 1.0)

            # Prioritized input loads, all on the sync HWDGE ring (FIFO).
            # The startup-critical chain is wk_ab -> eT[0:256] (first K unit)
            # and hT[0:512] (first two Q units).
            eT_v = eTd.rearrange("p (c q) -> p c q", c=NKC)
            hT_v = hTd.rearrange("p (c q) -> p c q", c=NKC)
            nc.sync.dma_start(out=wgtA_sb[:, 0:O_WQAB], in_=wgtA[:, 0:O_WQAB])
            nc.sync.dma_start(out=bias_sb, in_=biasf[:, :])
            nc.sync.dma_start(out=eT_sb[:, :, 0:256], in_=eT_v[:, :, 0:256])
            nc.sync.dma_start(out=hT_sb[:, :, 0:512], in_=hT_v[:, :, 0:512])
            nc.sync.dma_start(out=wgtA_sb[:, O_WQAB:NWA],
                              in_=wgtA[:, O_WQAB:NWA])
            nc.sync.dma_start(out=eT_sb[:, :, 256:768], in_=eT_v[:, :, 256:768])
            nc.sync.dma_start(out=eT_sb[:, :, 768:2048],
                              in_=eT_v[:, :, 768:2048])
            nc.sync.dma_start(out=wgtB_sb, in_=wgtB[:, :])
            nc.sync.dma_start(out=eT_sb[:, :, 2048:SKV],
                              in_=eT_v[:, :, 2048:SKV])
            nc.sync.dma_start(out=hT_sb[:, :, 512:SQ], in_=hT_v[:, :, 512:SQ])

            wk_ab_sb = wgtA_sb[:, O_WKAB:O_WQAB].rearrange("p (c m) -> p c m", c=NKC)
            wq_ab_sb = wgtA_sb[:, O_WQAB:O_WV3].rearrange("p (c m) -> p c m", c=NKC)
            wv3_sb = wgtA_sb[:, O_WV3:NWA].rearrange("p (c m) -> p c m", c=NKC)
            wq_2d_sb = wgtB_sb[:, O_WQ2D:O_WK2D].rearrange("p (c m) -> p c m", c=NKC)
            wk_2d_sb = wgtB_sb[:, O_WK2D:O_WOAB].rearrange("p (c m) -> p c m", c=NKC)
            wo_ab_sb = wgtB_sb[:, O_WOAB:O_WO2]
            wo_2_sb = wgtB_sb[:, O_WO2:NWB]
            bq_ab_sb = bias_sb[:, 0:1]
            bq_2d_sb = bias_sb[:, 1:2]
            bk_ab_sb = bias_sb[:, 2:3]
            bk_2d_sb = bias_sb[:, 3:4]
            bv_ab_sb = bias_sb[:, 4:5]
            bv_2_sb = bias_sb[:, 5:6]

            # One persistent aux PSUM bank, two 256-f32 slots ping-ponged by
            # every producer unit. Chains are sequential per bank (PE is
            # in-order); copies of slot s overlap the other slot's chain.
            auxt = auxp.tile([P, QC], F32, tag="aux")
            _slot_i = [0]

            def aux_slot(width):
                s = _slot_i[0] % 2
                _slot_i[0] += 1
                return auxt[:, s * 256:s * 256 + width]

            def proj_unit(dst, w_sb, b_sb, src, u):
                # dst[:, u*256:(u+1)*256] = (src chunks . w) + bias
                sl = aux_slot(256)
                for c in range(NKC):
                    nc.tensor.matmul(
                        sl, w_sb[:, c, :], src[:, c, u * 256:(u + 1) * 256],
                        start=(c == 0), stop=(c == NKC - 1),
                    )
                nc.vector.tensor_scalar_add(
                    dst[:, u * 256:(u + 1) * 256], sl, b_sb)

            next_k = [0]

            def emit_k(chunk):
                # K^T_ab units of 256 kv cols (2 chunks), pulled on demand
                while next_k[0] * 2 <= chunk + 2 and next_k[0] < SKV // 256:
                    proj_unit(kT_ab, wk_ab_sb, bk_ab_sb, eT_sb, next_k[0])
                    next_k[0] += 1

            next_v = [0]

            def emit_v(upto):
                # V projection, one kv-tile per slot
                while next_v[0] < min(upto + 1, NKV):
                    t = next_v[0]
                    sl = aux_slot(192)
                    for c in range(NKC):
                        nc.tensor.matmul(
                            sl, eT_sb[:, c, t * 128:(t + 1) * 128],
                            wv3_sb[:, c, :],
                            start=(c == 0), stop=(c == NKC - 1),
                        )
                    nc.vector.tensor_copy(
                        vv[:, t, :, 0:64],
                        sl.rearrange("p (h d) -> p h d", h=HPC),
                    )
                    next_v[0] = t + 1

            def finalize(ctx_t, dst_tile, dst_rows, bv_sb, q):
                # ctx_t[0:64] = unnormalized ctx^T; ctx_t[64] = softmax sums
                rc = finp.tile([1, QC], F32, tag="rc")
                nc.vector.reciprocal(rc, ctx_t[64:65, :])
                bc = finp.tile([64, QC], F32, tag="bc")
                nc.gpsimd.partition_broadcast(bc, rc)
                dst = dst_tile[dst_rows[0]:dst_rows[1], q * QC:(q + 1) * QC]
                nc.vector.tensor_mul(dst, ctx_t[0:64, :], bc)
                nc.vector.tensor_scalar_add(dst, dst, bv_sb)

            def attention_window(q, slices, jit, pull_k=False):
                """One attention window: scores+exp+PV for q-chunk q.

                jit: list of thunks (or None), one drained per mega.
                pull_k: emit kT_ab units on demand ahead of the score stream.
                """
                qs = slice(q * QC, (q + 1) * QC)
                nsl = len(slices)
                sizes, base = _mega_layout(nsl)
                ctxs = {}
                for h, c, rh in slices:
                    if h not in ctxs:
                        ctx_t = ctxp.tile([P, QC], F32, tag="ctx")
                        ctxs[h] = ctx_t

                def pv_mega(k, es_t):
                    emit_v(max(slices[base[k] + s][1] for s in range(sizes[k])))
                    for s in range(sizes[k]):
                        h, c, _ = slices[base[k] + s]
                        nc.tensor.matmul(
                            ctxs[h][0:65, :], vv[:, c, h, :],
                            es_t[:, s * QC:(s + 1) * QC],
                            start=(c == 0), stop=(c == NKV - 1),
                        )
                        if c == NKV - 1:
                            if h == 0:
                                finalize(ctxs[0], ctxn_ab, (0, 64),
                                         bv_ab_sb[0:64], q)
                            elif h == 1:
                                finalize(ctxs[1], ctxn_ab, (64, 128),
                                         bv_ab_sb[64:128], q)
                            else:
                                finalize(ctxs[2], ctxn_2, (0, 64),
                                         bv_2_sb[0:64], q)

                prev = None
                for m in range(len(sizes)):
                    if jit:
                        th = jit.pop(0)
                        if th is not None:
                            th()
                    sz = sizes[m]
                    if pull_k:
                        emit_k(max(slices[base[m] + s][1] for s in range(sz)))
                    if sz == 3:
                        st = sT3p.tile([P, 1536], F32, tag="st3")
                    else:
                        st = sT2p.tile([P, 1024], F32, tag="st2")
                    for s in range(sz):
                        h, c, rh = slices[base[m] + s]
                        kt, qt = (kT_ab, qT_ab) if h < 2 else (kT_2d, qT_2d)
                        if h < 2:
                            r0 = h * 64
                        else:
                            r0 = rh * 64
                        nc.tensor.matmul(
                            st[:, s * QC:(s + 1) * QC],
                            kt[r0:r0 + 64, c * 128:(c + 1) * 128],
                            qt[r0:r0 + 64, qs],
                            start=True, stop=True,
                        )
                    es_t = esp.tile([P, sz * QC], BF16,
                                    tag=("es3" if sz == 3 else "es2"))
                    nc.scalar.activation(es_t[:, 0:sz * QC], st[:, 0:sz * QC],
                                         AF.Exp, scale=0.125)
                    if prev is not None:
                        pv_mega(*prev)
                    prev = (m, es_t)
                pv_mega(*prev)
                while jit:
                    th = jit.pop(0)
                    if th is not None:
                        th()

            def proj_qtile(q, t):
                # output projection for one 128-q tile, 192-wide quarters
                # through the aux slots
                qoff = q * QC + t * 128
                ost = ostp.tile([P, D], F32, tag="ost")
                for n in range(4):
                    ns = slice(n * 192, (n + 1) * 192)
                    sl = aux_slot(192)
                    nc.tensor.matmul(
                        sl, ctxn_ab[:, qoff:qoff + 128],
                        wo_ab_sb[:, ns], start=True, stop=False,
                    )
                    nc.tensor.matmul(
                        sl, ctxn_2[0:64, qoff:qoff + 128],
                        wo_2_sb[0:64, ns], start=False, stop=True,
                    )
                    nc.vector.tensor_copy(ost[:, ns], sl)
                nc.sync.dma_start(out=out[qoff:qoff + 128, :], in_=ost)

            def pu(dst, w, b, src, u):
                return lambda: proj_unit(dst, w, b, src, u)

            def paced(thunks, n_megas, lead=0):
                # spread thunks over n_megas mega slots, starting at `lead`
                lst = [None] * n_megas
                if not thunks:
                    return lst
                step = max(1, (n_megas - lead) // len(thunks))
                i = lead
                for th in thunks:
                    while i < n_megas and lst[i] is not None:
                        i += 1
                    if i >= n_megas:
                        lst.append(th)
                    else:
                        lst[i] = th
                    i += step
                return lst

            # --- pre-attention: first K^T unit, Q^T units for qc0 ---
            emit_k(0)
            proj_unit(qT_ab, wq_ab_sb, bq_ab_sb, hT_sb, 0)
            proj_unit(qT_ab, wq_ab_sb, bq_ab_sb, hT_sb, 1)

            NM_A = len(_mega_layout(64)[0])   # 26
            NM_B = len(_mega_layout(32)[0])   # 13

            # Phase A: heads h0/h1 for all q-chunks. kT_ab and V are pulled
            # on demand; Q^T_ab for the next chunk and kT_2d / qT_2d drain on
            # a paced schedule.
            for q in range(NQC):
                th = []
                if q < NQC - 1:
                    th += [pu(qT_ab, wq_ab_sb, bq_ab_sb, hT_sb,
                              2 * (q + 1) + i) for i in range(2)]
                if q == 1:
                    th += [pu(kT_2d, wk_2d_sb, bk_2d_sb, eT_sb, g)
                           for g in range(8)]
                elif q == 2:
                    th += [pu(kT_2d, wk_2d_sb, bk_2d_sb, eT_sb, g)
                           for g in range(8, 16)]
                elif q == 3:
                    th += [pu(qT_2d, wq_2d_sb, bq_2d_sb, hT_sb, i)
                           for i in range(2)]
                lead = 8 if q == 0 else 0
                attention_window(q, SLICES_A, paced(th, NM_A, lead),
                                 pull_k=True)

            # Phase B: head h2 per q-chunk; the previous chunk's projection
            # drains per-qtile into the next window's PE slack.
            for q in range(NQC):
                th = []
                if q < NQC - 1:
                    th += [pu(qT_2d, wq_2d_sb, bq_2d_sb, hT_sb,
                              2 * (q + 1) + i) for i in range(2)]
                if q > 0:
                    th += [(lambda qq, tt: lambda: proj_qtile(qq, tt))(q - 1, t)
                           for t in range(4)]
                attention_window(q, SLICES_B, paced(th, NM_B))
            for t in range(4):
                proj_qtile(NQC - 1, t)
    nc.finalize()
    return nc


def _bf16(x):
    return np.ascontiguousarray(x.astype(ml_dtypes.bfloat16))


def _pack(w):
    # [768, M] -> [128, 6*M]  (partition-major view of 6 contract chunks)
    m = w.shape[1]
    return w.reshape(NKC, P, m).transpose(1, 0, 2).reshape(P, NKC * m)


def kernel(hidden_states, encoder_hidden_states, Wq, bq, Wk, bk, Wv, bv, Wo, bo):
    global LAST_RESULT, _CACHED_NC
    hidden_states = np.asarray(hidden_states, np.float32)
    encoder_hidden_states = np.asarray(encoder_hidden_states, np.float32)
    Wq, bq = np.asarray(Wq, np.float32), np.asarray(bq, np.float32)
    Wk, bk = np.asarray(Wk, np.float32), np.asarray(bk, np.float32)
    Wv, bv = np.asarray(Wv, np.float32), np.asarray(bv, np.float32)
    Wo, bo = np.asarray(Wo, np.float32), np.asarray(bo, np.float32)

    if _CACHED_NC is None:
        _CACHED_NC = _build_nc()
    nc = _CACHED_NC

    in_maps = []
    for core in range(8):
        b, g = divmod(core, 4)
        h0, h1, h2 = 3 * g, 3 * g + 1, 3 * g + 2
        sl = [slice(DH * h, DH * (h + 1)) for h in (h0, h1, h2)]
        wgtA_np = _bf16(np.concatenate([
            _pack(np.concatenate([Wk[:, sl[0]], Wk[:, sl[1]]], 1)),  # wk_ab
            _pack(np.concatenate([Wq[:, sl[0]], Wq[:, sl[1]]], 1)),  # wq_ab
            _pack(np.concatenate([Wv[:, s] for s in sl], 1)),        # wv3
        ], axis=1))
        wgtB_np = _bf16(np.concatenate([
            _pack(np.concatenate([Wq[:, sl[2]], Wq[:, sl[2]]], 1)),  # wq_2d
            _pack(np.concatenate([Wk[:, sl[2]], Wk[:, sl[2]]], 1)),  # wk_2d
            np.concatenate([Wo[sl[0]], Wo[sl[1]]], 0),               # wo_ab
            np.concatenate([Wo[sl[2]], np.zeros((64, D), np.float32)], 0),
        ], axis=1))
        assert wgtA_np.shape == (P, NWA) and wgtB_np.shape == (P, NWB)
        bias_np = np.stack([
            np.concatenate([bq[sl[0]], bq[sl[1]]]),
            np.concatenate([bq[sl[2]], bq[sl[2]]]),
            np.concatenate([bk[sl[0]], bk[sl[1]]]),
            np.concatenate([bk[sl[2]], bk[sl[2]]]),
            np.concatenate([bv[sl[0]], bv[sl[1]]]),
            np.concatenate([bv[sl[2]], np.zeros(64, np.float32)]),
        ], axis=1).astype(np.float32)
        in_maps.append({
            "wgtA": wgtA_np,
            "wgtB": wgtB_np,
            "biasf": bias_np,
            "eT": _bf16(_pack(encoder_hidden_states[b].T.copy())),
            "hT": _bf16(_pack(hidden_states[b].T.copy())),
        })

    res = run_bass_kernel_spmd(nc, in_maps, list(range(8)))
    LAST_RESULT = res

    outp = np.zeros((B, SQ, D), np.float32)
    for core in range(8):
        b = core // 4
        outp[b] += res.results[core]["out"]
    outp += bo
    return outp


# revision 21
# speedup vs baseline: 1.0062x; 1.0062x over previous
"""Trainium2 Bass kernel for CrossAttention (B=2, Sq=2048, Skv=4096, D=768, H=12).

Sharding: 8 cores = 2 batches x 4 head-groups (3 heads each). Each core
computes its 3 heads' attention and a partial output projection; the host
sums the 4 partial projections per batch and adds bo.

Device data flow (per core, matmul inputs bf16, accumulation fp32):
  - hidden^T / encoder^T shipped pre-transposed; DMAs ordered so the score
    pipeline starts as early as possible (wk_ab first, then the leading
    eT/hT columns).
  - Q^T/K^T computed in [dh(part), seq(free)] layout. Heads h0,h1 stacked on
    partition halves; h2 duplicated on both halves (weights shipped
    duplicated).
  - Scores S^T[kv, q] per 128-kv chunk, grouped into alternating
    [128,1536]/[128,1024] PSUM megas so ScalarE exp amortizes its startup.
  - All projection/producer work (K^T, Q^T, V, output projection) runs
    through two 256-element slots of a single persistent aux PSUM bank,
    ping-ponged so each unit's PSUM->SBUF copy overlaps the next unit's
    matmuls (chains stay sequential per bank; reads are unaffected by a
    later chain's start).
  - K^T_ab and V are produced on demand (pulled by the score/PV stream);
    the rest drains on a paced static schedule inside the windows.
  - PV: V augmented with a ones column (M=65) so PSUM accumulates ctx^T and
    the softmax denominator in one stream; PV issue deferred one mega so
    exp never waits behind PV in the PE queue.
  - Phase A slices stagger h1 one chunk behind h0 so the two finalizes
    (reciprocal + gpsimd broadcast + multiply) don't collide at the window
    boundary.
  - Output projection: ctx^T is exactly the lhsT needed; 192-wide quarters
    through the aux slots; partial out to DRAM.
"""

import numpy as np
import ml_dtypes

import concourse.bass as bass
import concourse.bacc as bacc
import concourse.mybir as mybir
from concourse.tile import TileContext
from concourse.bass_utils import run_bass_kernel_spmd

BF16 = mybir.dt.bfloat16
F32 = mybir.dt.float32
AF = mybir.ActivationFunctionType

B, SQ, SKV, D, H, DH = 2, 2048, 2048 * 2, 768, 12, 64
HPC = 3          # heads per core
NKC = 6          # contract chunks (768 / 128)
NKV = 32         # kv chunks (4096 / 128)
NQC = 4          # q chunks of 512
QC = 512
P = 128

# wgtA free-dim element offsets (bf16): weights needed first
O_WKAB = 0
O_WQAB = 768
O_WV3 = 1536
NWA = O_WV3 + NKC * 192      # 2688
# wgtB offsets: the rest
O_WQ2D = 0
O_WK2D = 768
O_WOAB = 1536
O_WO2 = O_WOAB + 768
NWB = O_WO2 + 768            # 3072
NE = NKC * SKV               # 24576 encoder^T
NH = NKC * SQ                # 12288 hidden^T


def _mega_layout(n):
    # alternating 3/2-slice megas covering n slices
    sizes = []
    left = n
    while left > 0:
        s = 3 if (len(sizes) % 2 == 0) else 2
        s = min(s, left)
        sizes.append(s)
        left -= s
    base = [0]
    for s in sizes:
        base.append(base[-1] + s)
    return sizes, base


LAST_RESULT = None  # BassKernelResults of the most recent run (for test.py)

_CACHED_NC = None

# Phase A slices: h0 runs one chunk ahead of h1 so their finalizes stagger.
SLICES_A = []
for _c in range(NKV):
    SLICES_A.append((0, _c, 0))
    if _c >= 1:
        SLICES_A.append((1, _c - 1, 1))
SLICES_A.append((1, NKV - 1, 1))
assert len(SLICES_A) == 64

# Phase B slices: head 2, chunk i, row-half alternating (duplicated tiles)
SLICES_B = [(2, _c, _c % 2) for _c in range(NKV)]


def _build_nc():
    nc = bacc.Bacc()

    wgtA = nc.declare_dram_parameter("wgtA", [P, NWA], BF16, isOutput=False)
    wgtB = nc.declare_dram_parameter("wgtB", [P, NWB], BF16, isOutput=False)
    biasf = nc.declare_dram_parameter("biasf", [P, 6], F32, isOutput=False)
    eTd = nc.declare_dram_parameter("eT", [P, NE], BF16, isOutput=False)
    hTd = nc.declare_dram_parameter("hT", [P, NH], BF16, isOutput=False)
    out = nc.declare_dram_parameter("out", [SQ, D], F32, isOutput=True)

    with TileContext(nc) as tc:
        with (
            tc.tile_pool(name="persist", bufs=1) as pp,
            tc.tile_pool(name="inb", bufs=1) as ip,
            tc.tile_pool(name="aux", bufs=1, space="PSUM") as auxp,
            tc.tile_pool(name="sT3", bufs=1, space="PSUM") as sT3p,
            tc.tile_pool(name="sT2", bufs=1, space="PSUM") as sT2p,
            tc.tile_pool(name="ctx", bufs=2, space="PSUM") as ctxp,
            tc.tile_pool(name="es", bufs=8) as esp,
            tc.tile_pool(name="fin", bufs=4) as finp,
            tc.tile_pool(name="ost", bufs=2) as ostp,
        ):
            qT_ab = pp.tile([P, SQ], BF16, tag="qT_ab")
            qT_2d = pp.tile([P, SQ], BF16, tag="qT_2d")
            kT_ab = pp.tile([P, SKV], BF16, tag="kT_ab")
            kT_2d = pp.tile([P, SKV], BF16, tag="kT_2d")
            vv = pp.tile([P, NKV, HPC, 65], BF16, tag="vv")
            ctxn_ab = pp.tile([P, SQ], BF16, tag="ctxn_ab")
            ctxn_2 = pp.tile([P, SQ], BF16, tag="ctxn_2")
            wgtA_sb = pp.tile([P, NWA], BF16, tag="wgtA")
            wgtB_sb = pp.tile([P, NWB], BF16, tag="wgtB")
            bias_sb = pp.tile([P, 6], F32, tag="biasf")
            warm = pp.tile([1, 2], F32, tag="warm")
            eT_sb = ip.tile([P, NKC, SKV], BF16, tag="eT")
            hT_sb = ip.tile([P, NKC, SQ], BF16, tag="hT")

            # Warm up the ScalarE exp table while DMAs run.
            nc.vector.memset(warm[0:1, 0:1], 0.0)
            nc.scalar.activation(warm[0:1, 1:2], warm[0:1, 0:1], AF.Exp)
            nc.vector.memset(vv[:, :, :, 64:65], 1.0)

            # Prioritized input loads, all on the sync HWDGE ring (FIFO).
            # The startup-critical chain is wk_ab -> eT[0:256] (first K unit)
            # and hT[0:512] (first two Q units).
            eT_v = eTd.rearrange("p (c q) -> p c q", c=NKC)
            hT_v = hTd.rearrange("p (c q) -> p c q", c=NKC)
            nc.sync.dma_start(out=wgtA_sb[:, 0:O_WQAB], in_=wgtA[:, 0:O_WQAB])
            nc.sync.dma_start(out=bias_sb, in_=biasf[:, :])
            nc.sync.dma_start(out=eT_sb[:, :, 0:256], in_=eT_v[:, :, 0:256])
            nc.sync.dma_start(out=hT_sb[:, :, 0:512], in_=hT_v[:, :, 0:512])
            nc.sync.dma_start(out=wgtA_sb[:, O_WQAB:NWA],
                              in_=wgtA[:, O_WQAB:NWA])
            nc.sync.dma_start(out=eT_sb[:, :, 256:768], in_=eT_v[:, :, 256:768])
            nc.sync.dma_start(out=eT_sb[:, :, 768:1280],
                              in_=eT_v[:, :, 768:1280])
            nc.sync.dma_start(out=eT_sb[:, :, 1280:2048],
                              in_=eT_v[:, :, 1280:2048])
            nc.sync.dma_start(out=wgtB_sb, in_=wgtB[:, :])
            nc.sync.dma_start(out=eT_sb[:, :, 2048:3072],
                              in_=eT_v[:, :, 2048:3072])
            nc.sync.dma_start(out=eT_sb[:, :, 3072:SKV],
                              in_=eT_v[:, :, 3072:SKV])
            nc.sync.dma_start(out=hT_sb[:, :, 512:SQ], in_=hT_v[:, :, 512:SQ])

            wk_ab_sb = wgtA_sb[:, O_WKAB:O_WQAB].rearrange("p (c m) -> p c m", c=NKC)
            wq_ab_sb = wgtA_sb[:, O_WQAB:O_WV3].rearrange("p (c m) -> p c m", c=NKC)
            wv3_sb = wgtA_sb[:, O_WV3:NWA].rearrange("p (c m) -> p c m", c=NKC)
            wq_2d_sb = wgtB_sb[:, O_WQ2D:O_WK2D].rearrange("p (c m) -> p c m", c=NKC)
            wk_2d_sb = wgtB_sb[:, O_WK2D:O_WOAB].rearrange("p (c m) -> p c m", c=NKC)
            wo_ab_sb = wgtB_sb[:, O_WOAB:O_WO2]
            wo_2_sb = wgtB_sb[:, O_WO2:NWB]
            bq_ab_sb = bias_sb[:, 0:1]
            bq_2d_sb = bias_sb[:, 1:2]
            bk_ab_sb = bias_sb[:, 2:3]
            bk_2d_sb = bias_sb[:, 3:4]
            bv_ab_sb = bias_sb[:, 4:5]
            bv_2_sb = bias_sb[:, 5:6]

            # One persistent aux PSUM bank, two 256-f32 slots ping-ponged by
            # every producer unit. Chains are sequential per bank (PE is
            # in-order); copies of slot s overlap the other slot's chain.
            auxt = auxp.tile([P, QC], F32, tag="aux")
            _slot_i = [0]

            def aux_slot(width):
                s = _slot_i[0] % 2
                _slot_i[0] += 1
                return auxt[:, s * 256:s * 256 + width]

            def proj_unit(dst, w_sb, b_sb, src, u):
                # dst[:, u*256:(u+1)*256] = (src chunks . w) + bias
                sl = aux_slot(256)
                for c in range(NKC):
                    nc.tensor.matmul(
                        sl, w_sb[:, c, :], src[:, c, u * 256:(u + 1) * 256],
                        start=(c == 0), stop=(c == NKC - 1),
                    )
                nc.vector.tensor_scalar_add(
                    dst[:, u * 256:(u + 1) * 256], sl, b_sb)

            next_k = [0]

            def emit_k(chunk):
                # K^T_ab units of 256 kv cols (2 chunks), pulled one mega
                # ahead of need (so the unit's PSUM->SBUF copy lands before
                # the consuming scores) but no further (an early unit would
                # block the in-order PE queue on its eT DMA).
                while next_k[0] <= (chunk + 2) // 2 and next_k[0] < SKV // 256:
                    proj_unit(kT_ab, wk_ab_sb, bk_ab_sb, eT_sb, next_k[0])
                    next_k[0] += 1

            next_v = [0]

            def emit_v(upto):
                # V projection, one kv-tile per slot
                while next_v[0] < min(upto + 1, NKV):
                    t = next_v[0]
                    sl = aux_slot(192)
                    for c in range(NKC):
                        nc.tensor.matmul(
                            sl, eT_sb[:, c, t * 128:(t + 1) * 128],
                            wv3_sb[:, c, :],
                            start=(c == 0), stop=(c == NKC - 1),
                        )
                    nc.vector.tensor_copy(
                        vv[:, t, :, 0:64],
                        sl.rearrange("p (h d) -> p h d", h=HPC),
                    )
                    next_v[0] = t + 1

            def finalize(ctx_t, dst_tile, dst_rows, bv_sb, q):
                # ctx_t[0:64] = unnormalized ctx^T; ctx_t[64] = softmax sums
                rc = finp.tile([1, QC], F32, tag="rc")
                nc.vector.reciprocal(rc, ctx_t[64:65, :])
                bc = finp.tile([64, QC], F32, tag="bc")
                nc.gpsimd.partition_broadcast(bc, rc)
                dst = dst_tile[dst_rows[0]:dst_rows[1], q * QC:(q + 1) * QC]
                nc.vector.tensor_mul(dst, ctx_t[0:64, :], bc)
                nc.vector.tensor_scalar_add(dst, dst, bv_sb)

            def attention_window(q, slices, jit, pull_k=False):
                """One attention window: scores+exp+PV for q-chunk q.

                jit: list of thunks (or None), one drained per mega.
                pull_k: emit kT_ab units on demand ahead of the score stream.
                """
                qs = slice(q * QC, (q + 1) * QC)
                nsl = len(slices)
                sizes, base = _mega_layout(nsl)
                ctxs = {}
                for h, c, rh in slices:
                    if h not in ctxs:
                        ctx_t = ctxp.tile([P, QC], F32, tag="ctx")
                        ctxs[h] = ctx_t

                def pv_mega(k, es_t):
                    emit_v(2 + max(slices[base[k] + s][1]
                                   for s in range(sizes[k])))
                    for s in range(sizes[k]):
                        h, c, _ = slices[base[k] + s]
                        nc.tensor.matmul(
                            ctxs[h][0:65, :], vv[:, c, h, :],
                            es_t[:, s * QC:(s + 1) * QC],
                            start=(c == 0), stop=(c == NKV - 1),
                        )
                        if c == NKV - 1:
                            if h == 0:
                                finalize(ctxs[0], ctxn_ab, (0, 64),
                                         bv_ab_sb[0:64], q)
                            elif h == 1:
                                finalize(ctxs[1], ctxn_ab, (64, 128),
                                         bv_ab_sb[64:128], q)
                            else:
                                finalize(ctxs[2], ctxn_2, (0, 64),
                                         bv_2_sb[0:64], q)

                prev = None
                for m in range(len(sizes)):
                    if jit:
                        th = jit.pop(0)
                        if th is not None:
                            th()
                    sz = sizes[m]
                    if pull_k:
                        emit_k(max(slices[base[m] + s][1] for s in range(sz)))
                    if sz == 3:
                        st = sT3p.tile([P, 1536], F32, tag="st3")
                    else:
                        st = sT2p.tile([P, 1024], F32, tag="st2")
                    for s in range(sz):
                        h, c, rh = slices[base[m] + s]
                        kt, qt = (kT_ab, qT_ab) if h < 2 else (kT_2d, qT_2d)
                        if h < 2:
                            r0 = h * 64
                        else:
                            r0 = rh * 64
                        nc.tensor.matmul(
                            st[:, s * QC:(s + 1) * QC],
                            kt[r0:r0 + 64, c * 128:(c + 1) * 128],
                            qt[r0:r0 + 64, qs],
                            start=True, stop=True,
                        )
                    es_t = esp.tile([P, sz * QC], BF16,
                                    tag=("es3" if sz == 3 else "es2"))
                    nc.scalar.activation(es_t[:, 0:sz * QC], st[:, 0:sz * QC],
                                         AF.Exp, scale=0.125)
                    if prev is not None:
                        pv_mega(*prev)
                    prev = (m, es_t)
                pv_mega(*prev)
                while jit:
                    th = jit.pop(0)
                    if th is not None:
                        th()

            def proj_qtile(q, t):
                # output projection for one 128-q tile, 192-wide quarters
                # through the aux slots
                qoff = q * QC + t * 128
                ost = ostp.tile([P, D], F32, tag="ost")
                for n in range(4):
                    ns = slice(n * 192, (n + 1) * 192)
                    sl = aux_slot(192)
                    nc.tensor.matmul(
                        sl, ctxn_ab[:, qoff:qoff + 128],
                        wo_ab_sb[:, ns], start=True, stop=False,
                    )
                    nc.tensor.matmul(
                        sl, ctxn_2[0:64, qoff:qoff + 128],
                        wo_2_sb[0:64, ns], start=False, stop=True,
                    )
                    nc.vector.tensor_copy(ost[:, ns], sl)
                nc.sync.dma_start(out=out[qoff:qoff + 128, :], in_=ost)

            def pu(dst, w, b, src, u):
                return lambda: proj_unit(dst, w, b, src, u)

            def paced(thunks, n_megas, lead=0):
                # spread thunks over n_megas mega slots, starting at `lead`
                lst = [None] * n_megas
                if not thunks:
                    return lst
                step = max(1, (n_megas - lead) // len(thunks))
                i = lead
                for th in thunks:
                    while i < n_megas and lst[i] is not None:
                        i += 1
                    if i >= n_megas:
                        lst.append(th)
                    else:
                        lst[i] = th
                    i += step
                return lst

            # --- pre-attention: first K^T unit, Q^T units for qc0 ---
            emit_k(0)
            proj_unit(qT_ab, wq_ab_sb, bq_ab_sb, hT_sb, 0)
            proj_unit(qT_ab, wq_ab_sb, bq_ab_sb, hT_sb, 1)

            NM_A = len(_mega_layout(64)[0])   # 26
            NM_B = len(_mega_layout(32)[0])   # 13

            # Phase A: heads h0/h1 for all q-chunks. kT_ab and V are pulled
            # on demand; Q^T_ab for the next chunk and kT_2d / qT_2d drain on
            # a paced schedule.
            for q in range(NQC):
                th = []
                if q < NQC - 1:
                    th += [pu(qT_ab, wq_ab_sb, bq_ab_sb, hT_sb,
                              2 * (q + 1) + i) for i in range(2)]
                if q == 1:
                    th += [pu(kT_2d, wk_2d_sb, bk_2d_sb, eT_sb, g)
                           for g in range(5)]
                elif q == 2:
                    th += [pu(kT_2d, wk_2d_sb, bk_2d_sb, eT_sb, g)
                           for g in range(5, 11)]
                elif q == 3:
                    th += [pu(kT_2d, wk_2d_sb, bk_2d_sb, eT_sb, g)
                           for g in range(11, 15)]
                    th += [pu(qT_2d, wq_2d_sb, bq_2d_sb, hT_sb, i)
                           for i in range(2)]
                lead = 8 if q == 0 else 0
                attention_window(q, SLICES_A, paced(th, NM_A, lead),
                                 pull_k=True)

            # Phase B: head h2 per q-chunk; the previous chunk's projection
            # drains per-qtile into the next window's PE slack.
            for q in range(NQC):
                th = []
                if q == 0:
                    th += [pu(kT_2d, wk_2d_sb, bk_2d_sb, eT_sb, 15)]
                if q < NQC - 1:
                    th += [pu(qT_2d, wq_2d_sb, bq_2d_sb, hT_sb,
                              2 * (q + 1) + i) for i in range(2)]
                if q > 0:
                    th += [(lambda qq, tt: lambda: proj_qtile(qq, tt))(q - 1, t)
                           for t in range(4)]
                attention_window(q, SLICES_B, paced(th, NM_B))
            for t in range(4):
                proj_qtile(NQC - 1, t)
    nc.finalize()
    return nc


def _bf16(x):
    return np.ascontiguousarray(x.astype(ml_dtypes.bfloat16))


def _pack(w):
    # [768, M] -> [128, 6*M]  (partition-major view of 6 contract chunks)
    m = w.shape[1]
    return w.reshape(NKC, P, m).transpose(1, 0, 2).reshape(P, NKC * m)


def kernel(hidden_states, encoder_hidden_states, Wq, bq, Wk, bk, Wv, bv, Wo, bo):
    global LAST_RESULT, _CACHED_NC
    hidden_states = np.asarray(hidden_states, np.float32)
    encoder_hidden_states = np.asarray(encoder_hidden_states, np.float32)
    Wq, bq = np.asarray(Wq, np.float32), np.asarray(bq, np.float32)
    Wk, bk = np.asarray(Wk, np.float32), np.asarray(bk, np.float32)
    Wv, bv = np.asarray(Wv, np.float32), np.asarray(bv, np.float32)
    Wo, bo = np.asarray(Wo, np.float32), np.asarray(bo, np.float32)

    if _CACHED_NC is None:
        _CACHED_NC = _build_nc()
    nc = _CACHED_NC

    in_maps = []
    for core in range(8):
        b, g = divmod(core, 4)
        h0, h1, h2 = 3 * g, 3 * g + 1, 3 * g + 2
        sl = [slice(DH * h, DH * (h + 1)) for h in (h0, h1, h2)]
        wgtA_np = _bf16(np.concatenate([
            _pack(np.concatenate([Wk[:, sl[0]], Wk[:, sl[1]]], 1)),  # wk_ab
            _pack(np.concatenate([Wq[:, sl[0]], Wq[:, sl[1]]], 1)),  # wq_ab
            _pack(np.concatenate([Wv[:, s] for s in sl], 1)),        # wv3
        ], axis=1))
        wgtB_np = _bf16(np.concatenate([
            _pack(np.concatenate([Wq[:, sl[2]], Wq[:, sl[2]]], 1)),  # wq_2d
            _pack(np.concatenate([Wk[:, sl[2]], Wk[:, sl[2]]], 1)),  # wk_2d
            np.concatenate([Wo[sl[0]], Wo[sl[1]]], 0),               # wo_ab
            np.concatenate([Wo[sl[2]], np.zeros((64, D), np.float32)], 0),
        ], axis=1))
        assert wgtA_np.shape == (P, NWA) and wgtB_np.shape == (P, NWB)
        bias_np = np.stack([
            np.concatenate([bq[sl[0]], bq[sl[1]]]),
            np.concatenate([bq[sl[2]], bq[sl[2]]]),
            np.concatenate([bk[sl[0]], bk[sl[1]]]),
            np.concatenate([bk[sl[2]], bk[sl[2]]]),
            np.concatenate([bv[sl[0]], bv[sl[1]]]),
            np.concatenate([bv[sl[2]], np.zeros(64, np.float32)]),
        ], axis=1).astype(np.float32)
        in_maps.append({
            "wgtA": wgtA_np,
            "wgtB": wgtB_np,
            "biasf": bias_np,
            "eT": _bf16(_pack(encoder_hidden_states[b].T.copy())),
            "hT": _bf16(_pack(hidden_states[b].T.copy())),
        })

    res = run_bass_kernel_spmd(nc, in_maps, list(range(8)))
    LAST_RESULT = res

    outp = np.zeros((B, SQ, D), np.float32)
    for core in range(8):
        b = core // 4
        outp[b] += res.results[core]["out"]
    outp += bo
    return outp
